# revision 5
# baseline (speedup 1.0000x reference)
"""MultiHeadAttention TRN2 kernel: 8-way (batch x head-half) sharding.

Core c handles batch b=c//2, heads g*8..g*8+8 where g=c%2.
Per core: Q^T/K^T projections (lhsT=W-slice, rhs=X^T), V in natural layout
with a fused ones-column (softmax denominators fall out of the P@V matmul),
scores computed transposed (keys on partition -> key mask folds into the
per-partition ACT bias of the exp), PV matmul -> ctx^T, normalization fused
into the PSUM eviction, partial FC (row-slice of Wfc). The two half-head
partials per batch are summed on the host during unsharding.

All matmuls run in float32r (full PE rate at N>=256, ~1e-4 rel accuracy).
"""

import numpy as np

import concourse.mybir as mybir
import concourse.tile as tile
from concourse import bacc
from concourse.bass import ts
from concourse.bass_utils import run_bass_kernel_spmd

F32 = mybir.dt.float32
F32R = mybir.dt.float32r
AF = mybir.ActivationFunctionType

BS, L, D = 4, 2048, 1024
NCORES = 8
H = 8                 # heads per core
DK = 64
HD = H * DK           # 512: head dims per core
KC = L // 128         # 16 key chunks
QC = L // 512         # 4 query chunks
NEGB = -30000.0       # masked-key bias (exp underflows to exactly 0)
SCALE = 1.0 / 8.0     # 1/sqrt(DK)


def _build():
    nc = bacc.Bacc()
    xt = nc.declare_dram_parameter("xt", [8, 128, L], F32R, isOutput=False)
    wq = nc.declare_dram_parameter("wq", [8, 128, HD], F32R, isOutput=False)
    wk = nc.declare_dram_parameter("wk", [8, 128, HD], F32R, isOutput=False)
    wv = nc.declare_dram_parameter("wv", [8, 128, HD], F32R, isOutput=False)
    wfc = nc.declare_dram_parameter("wfc", [4, 128, D], F32R, isOutput=False)
    bq = nc.declare_dram_parameter("bq", [4, 128, 1], F32, isOutput=False)
    bk = nc.declare_dram_parameter("bk", [4, 128, 1], F32, isOutput=False)
    bvr = nc.declare_dram_parameter("bvr", [1, HD], F32R, isOutput=False)
    bfch = nc.declare_dram_parameter("bfch", [8, 128, 1], F32, isOutput=False)
    mb = nc.declare_dram_parameter("mb", [128, KC], F32, isOutput=False)
    outp = nc.declare_dram_parameter("out", [8, 128, L], F32, isOutput=True)

    with tile.TileContext(nc) as tc:
        with tc.tile_pool(name="const", bufs=1) as pc, \
             tc.tile_pool(name="qt", bufs=4) as p_qt, \
             tc.tile_pool(name="kt", bufs=4) as p_kt, \
             tc.tile_pool(name="v", bufs=16) as p_v:
            # constants
            ones_f = pc.tile([1, 128], F32)
            nc.vector.memset(ones_f[:], 1.0)
            ones_r = pc.tile([1, 128], F32R)
            nc.vector.tensor_copy(ones_r[:], ones_f[:])
            onesv = pc.tile([128, 8, 1], F32)
            nc.vector.memset(onesv[:], 1.0)
            mb_sb = pc.tile([128, KC], F32)
            nc.sync.dma_start(out=mb_sb[:], in_=mb[:])
            bv_sb = pc.tile([1, HD], F32R)
            nc.sync.dma_start(out=bv_sb[:], in_=bvr[:])

            qt_t = [p_qt.tile([128, L], F32R, tag="qt", name=f"qt{i}") for i in range(4)]
            kt_t = [p_kt.tile([128, L], F32R, tag="kt", name=f"kt{i}") for i in range(4)]
            v_t = [p_v.tile([128, 8, 65], F32R, tag="v", name=f"v{i}") for i in range(KC)]

            # ---------------- Phase A: projections ----------------
            with tc.tile_pool(name="xt", bufs=8) as p_xt, \
                 tc.tile_pool(name="wchunk", bufs=12) as p_w, \
                 tc.tile_pool(name="wvp", bufs=8) as p_wv, \
                 tc.tile_pool(name="biasA", bufs=4) as p_b, \
                 tc.tile_pool(name="psA", bufs=4, space="PSUM") as psA:
                xt_t = []
                for k in range(8):
                    t = p_xt.tile([128, L], F32R, tag="xt")
                    nc.sync.dma_start(out=t[:], in_=xt[k])
                    xt_t.append(t)

                # Q^T and K^T: out rows t*128, cols = tokens
                for t in range(4):
                    wq_c, wk_c = [], []
                    for k in range(8):
                        cq = p_w.tile([128, 128], F32R, tag="wc")
                        nc.sync.dma_start(out=cq[:], in_=wq[k][:, ts(t, 128)])
                        wq_c.append(cq)
                    for k in range(8):
                        ck = p_w.tile([128, 128], F32R, tag="wc")
                        nc.sync.dma_start(out=ck[:], in_=wk[k][:, ts(t, 128)])
                        wk_c.append(ck)
                    bq_t = p_b.tile([128, 1], F32, tag="b")
                    nc.sync.dma_start(out=bq_t[:], in_=bq[t])
                    bk_t = p_b.tile([128, 1], F32, tag="b")
                    nc.sync.dma_start(out=bk_t[:], in_=bk[t])
                    for n in range(QC):
                        ps = psA.tile([128, 512], F32, tag="mm")
                        for k in range(8):
                            nc.tensor.matmul(ps[:], wq_c[k][:], xt_t[k][:, ts(n, 512)],
                                             start=(k == 0), stop=(k == 7))
                        nc.vector.tensor_scalar_add(qt_t[t][:, ts(n, 512)], ps[:], bq_t[:])
                        ps = psA.tile([128, 512], F32, tag="mm")
                        for k in range(8):
                            nc.tensor.matmul(ps[:], wk_c[k][:], xt_t[k][:, ts(n, 512)],
                                             start=(k == 0), stop=(k == 7))
                        nc.vector.tensor_scalar_add(kt_t[t][:, ts(n, 512)], ps[:], bk_t[:])

                # V natural layout (tokens on partition), + bias via K=1 matmul
                wv_c = []
                for k in range(8):
                    cv = p_wv.tile([128, HD], F32R, tag="wv")
                    nc.sync.dma_start(out=cv[:], in_=wv[k])
                    wv_c.append(cv)
                for m in range(KC):
                    ps = psA.tile([128, 512], F32, tag="mm")
                    for k in range(8):
                        nc.tensor.matmul(ps[:], xt_t[k][:, ts(m, 128)], wv_c[k][:],
                                         start=(k == 0), stop=False)
                    nc.tensor.matmul(ps[:], ones_r[:, :128], bv_sb[:],
                                     start=False, stop=True)
                    nc.vector.tensor_copy(
                        v_t[m][:, :, 0:64],
                        ps[:].rearrange("p (h d) -> p h d", h=8))
                    nc.vector.tensor_copy(v_t[m][:, :, 64:65], onesv[:])

            # ---------------- Phase B: attention ----------------
            with tc.tile_pool(name="ctx", bufs=4) as p_ctx:
                ctx_t = [p_ctx.tile([128, L], F32R, tag="ctx", name=f"ctx{i}") for i in range(4)]
                with tc.tile_pool(name="pt", bufs=20) as p_pt, \
                     tc.tile_pool(name="smallB", bufs=4) as p_sm, \
                     tc.tile_pool(name="psS", bufs=4, space="PSUM") as psS, \
                     tc.tile_pool(name="psC", bufs=2, space="PSUM") as psCtx, \
                     tc.tile_pool(name="psR", bufs=2, space="PSUM") as psR:
                    for h in range(H):
                        th, oh = h // 2, (h % 2) * 64
                        for q in range(QC):
                            pts = []
                            for kc in range(KC):
                                sps = psS.tile([128, 512], F32, tag="s")
                                nc.tensor.matmul(
                                    sps[:],
                                    kt_t[th][oh:oh + 64, ts(kc, 128)],
                                    qt_t[th][oh:oh + 64, ts(q, 512)],
                                    start=True, stop=True)
                                pt = p_pt.tile([128, 512], F32R, tag="pt")
                                nc.scalar.activation(pt[:], sps[:], AF.Exp,
                                                     bias=mb_sb[:, kc:kc + 1],
                                                     scale=SCALE)
                                pts.append(pt)
                            cps = psCtx.tile([65, 512], F32, tag="ctxp")
                            for kc in range(KC):
                                nc.tensor.matmul(cps[:], v_t[kc][:, h, :], pts[kc][:],
                                                 start=(kc == 0), stop=(kc == KC - 1))
                            recip = p_sm.tile([1, 512], F32, tag="recip")
                            nc.vector.reciprocal(recip[:], cps[64:65, :])
                            recr = p_sm.tile([1, 512], F32R, tag="recr")
                            nc.vector.tensor_copy(recr[:], recip[:])
                            rbps = psR.tile([64, 512], F32, tag="rb")
                            nc.tensor.matmul(rbps[:], ones_r[:, 0:64], recr[:],
                                             start=True, stop=True)
                            rbs = p_sm.tile([64, 512], F32, tag="rbs")
                            nc.vector.tensor_copy(rbs[:], rbps[:])
                            nc.vector.tensor_mul(
                                ctx_t[th][oh:oh + 64, ts(q, 512)],
                                cps[0:64, :], rbs[:])

                # ---------------- Phase C: fc partial ----------------
                with tc.tile_pool(name="wfc", bufs=4) as p_wfc, \
                     tc.tile_pool(name="biasC", bufs=2) as p_bc, \
                     tc.tile_pool(name="ev", bufs=4) as p_ev, \
                     tc.tile_pool(name="psF", bufs=4, space="PSUM") as psF:
                    wfc_c = []
                    for k in range(4):
                        cf = p_wfc.tile([128, D], F32R, tag="wfc")
                        nc.sync.dma_start(out=cf[:], in_=wfc[k])
                        wfc_c.append(cf)
                    for m in range(8):
                        bfc_m = p_bc.tile([128, 1], F32, tag="bc")
                        nc.sync.dma_start(out=bfc_m[:], in_=bfch[m])
                        for n in range(QC):
                            ps = psF.tile([128, 512], F32, tag="f")
                            for k in range(4):
                                nc.tensor.matmul(ps[:], wfc_c[k][:, ts(m, 128)],
                                                 ctx_t[k][:, ts(n, 512)],
                                                 start=(k == 0), stop=(k == 3))
                            ev = p_ev.tile([128, 512], F32, tag="ev")
                            nc.vector.tensor_scalar_add(ev[:], ps[:], bfc_m[:])
                            nc.sync.dma_start(out=outp[m][:, ts(n, 512)], in_=ev[:])

    nc.finalize()
    return nc


class _Runner:
    """Compile-once wrapper around the run_bass_via_pjrt shard_map path."""

    def __init__(self, nc):
        import jax
        from jax.sharding import Mesh, PartitionSpec

        from concourse import bass2jax, mybir as mb

        try:
            from jax.experimental.shard_map import shard_map
        except ImportError:
            from jax.shard_map import shard_map

        bass2jax.install_neuronx_cc_hook()
        self._nc = nc
        partition_name = (nc.partition_id_tensor.name
                          if nc.partition_id_tensor else None)
        in_names, out_names, out_avals = [], [], []
        self._zero_shapes = []
        for alloc in nc.m.functions[0].allocations:
            if not isinstance(alloc, mb.MemoryLocationSet):
                continue
            name = alloc.memorylocations[0].name
            if alloc.kind == "ExternalInput":
                if name != partition_name:
                    in_names.append(name)
            elif alloc.kind == "ExternalOutput":
                out_names.append(name)
                shape = tuple(alloc.tensor_shape)
                dtype = mb.dt.np(alloc.dtype)
                out_avals.append(jax.core.ShapedArray(shape, dtype))
                self._zero_shapes.append((shape, dtype))
        self._n_params = len(in_names)
        n_outs = len(out_avals)
        self._in_names = list(in_names)
        self._out_names = list(out_names)
        self._out_avals = out_avals
        all_in = in_names + out_names
        if partition_name is not None:
            all_in.append(partition_name)

        def _body(*args):
            operands = list(args)
            if partition_name is not None:
                operands.append(bass2jax.partition_id_tensor())
            return tuple(bass2jax._bass_exec_p.bind(
                *operands,
                out_avals=tuple(out_avals),
                in_names=tuple(all_in),
                out_names=tuple(out_names),
                lowering_input_output_aliases=(),
                sim_require_finite=True,
                sim_require_nnan=True,
                nc=nc,
            ))

        devices = jax.devices()[:NCORES]
        mesh = Mesh(np.asarray(devices), ("core",))
        self.mesh = mesh
        nin = self._n_params + n_outs
        self._sharded = jax.jit(
            shard_map(_body, mesh=mesh,
                      in_specs=(PartitionSpec("core"),) * nin,
                      out_specs=(PartitionSpec("core"),) * n_outs,
                      check_rep=False),
            donate_argnums=tuple(range(self._n_params, nin)),
            keep_unused=True,
        )

    def run(self, in_maps):
        concat_in = [
            np.concatenate([np.asarray(in_maps[c][name])
                            for c in range(NCORES)], axis=0)
            for name in self._in_names
        ]
        concat_zeros = [np.zeros((NCORES * s[0], *s[1:]), d)
                        for s, d in self._zero_shapes]
        out_arrs = self._sharded(*concat_in, *concat_zeros)
        return [
            {name: np.asarray(out_arrs[i]).reshape(
                NCORES, *self._out_avals[i].shape)[c]
             for i, name in enumerate(self._out_names)}
            for c in range(NCORES)
        ]


_RUNNER = None


def _get_runner():
    global _RUNNER
    if _RUNNER is None:
        _RUNNER = _Runner(_build())
    return _RUNNER


def kernel(x, mask, Wq, bq, Wk, bk, Wv, bv, Wfc, bfc, **_unused):
    x = np.asarray(x, np.float32)
    mask = np.asarray(mask)
    Wq, bq = np.asarray(Wq, np.float32), np.asarray(bq, np.float32)
    Wk, bk = np.asarray(Wk, np.float32), np.asarray(bk, np.float32)
    Wv, bv = np.asarray(Wv, np.float32), np.asarray(bv, np.float32)
    Wfc, bfc = np.asarray(Wfc, np.float32), np.asarray(bfc, np.float32)

    in_maps = []
    for c in range(NCORES):
        b, g = c // 2, c % 2
        sl = slice(g * HD, (g + 1) * HD)
        xt_c = np.ascontiguousarray(x[b].T).reshape(8, 128, L)
        mb_c = np.ascontiguousarray(
            (mask[b].astype(np.float32) * NEGB).reshape(KC, 128).T)
        in_maps.append({
            "xt": xt_c,
            "wq": np.ascontiguousarray(Wq[:, sl]).reshape(8, 128, HD),
            "wk": np.ascontiguousarray(Wk[:, sl]).reshape(8, 128, HD),
            "wv": np.ascontiguousarray(Wv[:, sl]).reshape(8, 128, HD),
            "wfc": np.ascontiguousarray(Wfc[sl, :]).reshape(4, 128, D),
            "bq": np.ascontiguousarray(bq[sl]).reshape(4, 128, 1),
            "bk": np.ascontiguousarray(bk[sl]).reshape(4, 128, 1),
            "bvr": np.ascontiguousarray(bv[sl]).reshape(1, HD),
            "bfch": np.ascontiguousarray(bfc * 0.5).reshape(8, 128, 1),
            "mb": mb_c,
        })

    results = _get_runner().run(in_maps)

    out = np.empty((BS, L, D), np.float32)
    for b in range(BS):
        p0 = results[2 * b]["out"].reshape(D, L)
        p1 = results[2 * b + 1]["out"].reshape(D, L)
        out[b] = (p0 + p1).T
    return out
